# revision 1
# baseline (speedup 1.0000x reference)
"""NeuralODE Euler-integration kernel for 8 TRN2 NeuronCores.

Problem: y' = MLP(y) with MLP = Linear(64,256)+ReLU, Linear(256,256)+ReLU,
Linear(256,64); Euler steps y_{t+1} = y_t + dt*MLP(y_t), T=200 steps total
(199 integration steps), B=4096, output [B, 200, 64] with slot 0 = y0.

Sharding: pure data-parallel. Each of the 8 cores owns 512 batch rows.
Weights are replicated. No inter-core communication.

On-device design (per core):
  - State kept transposed: [64 features (partitions), 512 batch (free)],
    in TWO copies: fp32 master y32 (exact Euler accumulation) and a bf16
    shadow yr that feeds the next step's first matmul. Both are produced by
    parallel VectorE `scalar_tensor_tensor` ops reading the same PSUM bank:
        yr/y32 = (ps3 + dt*b3) + y32_prev
    (W3, b3 pre-scaled by dt on host, so ps3 = dt*W3^T h2.)
  - Weights pre-arranged on host so every matmul uses them as lhsT directly.
  - Matmuls run in bf16 (1 cycle/col at warm PE clock, fp32 accumulate);
    hidden activations h1/h2 are bf16 (rounding enters only the dt-damped
    derivative path; the fp32 master keeps state accumulation exact).
  - relu(x+b): split between ScalarE (activation bias) and VectorE
    (tensor_scalar add+max) so both hidden chunks materialize in parallel.
  - Batch processed as `nsub`=2 sub-batches of 256 pipelined so PE keeps
    streaming while ACT/DVE work on the other sub-batch.
  - Optional `nwarm` dummy matmuls per step raise PE duty so the HAM
    clock-gate holds 2.4 GHz.
  - Per step, y32 is DMA'd to HBM as out[t-1] ([199, 64, 512], t-major,
    feature-major, contiguous 2KB per partition). Host reassembles to
    [B, 200, 64].
"""
import numpy as np

import concourse.bass as bass
import concourse.tile as tile
from concourse import bacc, mybir
from concourse.bass_utils import run_bass_kernel_spmd

F32 = mybir.dt.float32
BF16 = mybir.dt.bfloat16
RELU = mybir.ActivationFunctionType.Relu

B, D, H, T = 4096, 64, 256, 200
NCORES = 8
BL = B // NCORES          # 512 batch rows per core

_cache = {}


def build(nsteps: int, nsub: int = 2, nwarm: int = 0,
          act_relu2c1: bool = True, merged_upd: bool = False):
    add = mybir.AluOpType.add
    mx = mybir.AluOpType.max
    FD = BL // nsub

    nc = bacc.Bacc("TRN2", target_bir_lowering=False, debug=False)
    y0r_d = nc.dram_tensor("y0Tr", [D, BL], BF16, kind="ExternalInput")
    y032_d = nc.dram_tensor("y0T32", [D, BL], F32, kind="ExternalInput")
    w1_d = nc.dram_tensor("w1", [D, 2, 128], BF16, kind="ExternalInput")
    w2_d = nc.dram_tensor("w2", [128, 2, 2, 128], BF16, kind="ExternalInput")
    w3_d = nc.dram_tensor("w3", [128, 2, D], BF16, kind="ExternalInput")
    b1_d = nc.dram_tensor("b1r", [128, 2], F32, kind="ExternalInput")
    b2_d = nc.dram_tensor("b2r", [128, 2], F32, kind="ExternalInput")
    b3_d = nc.dram_tensor("b3r", [D, 1], F32, kind="ExternalInput")
    out_d = nc.dram_tensor("out", [nsteps, D, BL], F32, kind="ExternalOutput")

    with tile.TileContext(nc) as tc:
        with tc.tile_pool(name="wpool", bufs=1) as wp, \
             tc.tile_pool(name="state", bufs=1) as sp, \
             tc.tile_pool(name="hpool", bufs=8) as hp, \
             tc.tile_pool(name="ps", bufs=(5 if merged_upd else 7), space="PSUM") as pp, \
             tc.tile_pool(name="warm", bufs=1, space="PSUM") as wpp:

            w1 = wp.tile([D, 2, 128], BF16)
            w2 = wp.tile([128, 2, 2, 128], BF16)
            w3 = wp.tile([128, 2, D], BF16)
            b1 = wp.tile([128, 2], F32)
            b2 = wp.tile([128, 2], F32)
            b3 = wp.tile([D, 1], F32)
            nc.sync.dma_start(w1[:], w1_d.ap())
            nc.sync.dma_start(w2[:], w2_d.ap())
            nc.sync.dma_start(w3[:], w3_d.ap())
            nc.sync.dma_start(b1[:], b1_d.ap())
            nc.sync.dma_start(b2[:], b2_d.ap())
            nc.sync.dma_start(b3[:], b3_d.ap())

            NR = 4
            yrs = [sp.tile([D, BL], BF16, tag=f"yr{i}", name=f"yr{i}")
                   for i in range(NR)]
            y32s = [sp.tile([D, BL], F32, tag=f"y32{i}", name=f"y32{i}")
                    for i in range(NR)]
            nc.sync.dma_start(yrs[0][:], y0r_d.ap())
            nc.sync.dma_start(y32s[0][:], y032_d.ap())

            if nwarm:
                wps = wpp.tile([128, 256], F32, name="warmps")

            out_ap = out_d.ap()

            if nwarm:
                for w in range(30):
                    nc.tensor.matmul(wps[:], w2[:, 0, 0, :],
                                     w2[:, 0, :, :].rearrange("p a b -> p (a b)"),
                                     start=True, stop=True, skip_group_check=True)

            for t in range(1, nsteps + 1):
                srcr = yrs[(t - 1) % NR]
                dstr = yrs[t % NR]
                src32 = y32s[(t - 1) % NR]
                dst32 = y32s[t % NR]

                for w in range(nwarm):
                    nc.tensor.matmul(wps[:], w2[:, 0, 0, :],
                                     w2[:, 0, :, :].rearrange("p a b -> p (a b)"),
                                     start=True, stop=True, skip_group_check=True)

                for s in range(nsub):
                    cs = bass.ts(s, FD)
                    # ---- layer 1 ----
                    ps1 = [pp.tile([128, FD], F32, tag="ps", name="ps1")
                           for _ in range(2)]
                    for mc in range(2):
                        nc.tensor.matmul(ps1[mc][:], w1[:, mc, :], srcr[:, cs],
                                         start=True, stop=True)
                    h1 = hp.tile([128, 2, FD], BF16, tag="h", name="h1")
                    nc.scalar.activation(h1[:, 0, :], ps1[0][:], RELU,
                                         bias=b1[:, 0:1], scale=1.0)
                    nc.vector.tensor_scalar(h1[:, 1, :], ps1[1][:],
                                            b1[:, 1:2], 0.0, op0=add, op1=mx)
                    # ---- layer 2 ----
                    ps2 = [pp.tile([128, FD], F32, tag="ps", name="ps2")
                           for _ in range(2)]
                    for mc in range(2):
                        for kc in range(2):
                            nc.tensor.matmul(ps2[mc][:], w2[:, kc, mc, :],
                                             h1[:, kc, :],
                                             start=(kc == 0), stop=(kc == 1))
                    h2 = hp.tile([128, 2, FD], BF16, tag="h", name="h2")
                    nc.scalar.activation(h2[:, 0, :], ps2[0][:], RELU,
                                         bias=b2[:, 0:1], scale=1.0)
                    if act_relu2c1:
                        nc.scalar.activation(h2[:, 1, :], ps2[1][:], RELU,
                                             bias=b2[:, 1:2], scale=1.0)
                    else:
                        nc.vector.tensor_scalar(h2[:, 1, :], ps2[1][:],
                                                b2[:, 1:2], 0.0, op0=add, op1=mx)
                    # ---- layer 3 + Euler updates ----
                    if merged_upd:
                        if s == 0:
                            ps3m = pp.tile([D, 2, FD], F32, tag="ps3m",
                                           name="ps3m", bufs=2)
                        for kc in range(2):
                            nc.tensor.matmul(ps3m[:, s, :], w3[:, kc, :],
                                             h2[:, kc, :],
                                             start=(kc == 0), stop=(kc == 1))
                        if s == nsub - 1:
                            pf = ps3m.rearrange("p a b -> p (a b)")
                            nc.vector.scalar_tensor_tensor(dstr[:], pf,
                                                           b3[:, 0:1], src32[:],
                                                           op0=add, op1=add)
                            nc.vector.scalar_tensor_tensor(dst32[:], pf,
                                                           b3[:, 0:1], src32[:],
                                                           op0=add, op1=add)
                    else:
                        ps3 = pp.tile([D, FD], F32, tag="ps", name="ps3")
                        for kc in range(2):
                            nc.tensor.matmul(ps3[:], w3[:, kc, :], h2[:, kc, :],
                                             start=(kc == 0), stop=(kc == 1))
                        # bf16 shadow feeds next step's matmul (critical path)
                        nc.vector.scalar_tensor_tensor(dstr[:, cs], ps3[:],
                                                       b3[:, 0:1], src32[:, cs],
                                                       op0=add, op1=add)
                        # fp32 master keeps exact Euler accumulation
                        nc.vector.scalar_tensor_tensor(dst32[:, cs], ps3[:],
                                                       b3[:, 0:1], src32[:, cs],
                                                       op0=add, op1=add)
                nc.sync.dma_start(out_ap[t - 1], dst32[:])
    nc.compile()
    return nc


def _prep_inputs(y0, t, W1, b1, W2, b2, W3, b3):
    import ml_dtypes
    bf16 = ml_dtypes.bfloat16
    dt = float(t[1] - t[0])
    w1r = np.ascontiguousarray(W1.reshape(D, 2, 128))
    w2r = np.ascontiguousarray(W2.reshape(2, 128, 2, 128).transpose(1, 0, 2, 3))
    w3r = np.ascontiguousarray((dt * W3).reshape(2, 128, D).transpose(1, 0, 2))
    b1r = np.ascontiguousarray(b1.reshape(2, 128).T)
    b2r = np.ascontiguousarray(b2.reshape(2, 128).T)
    b3r = np.ascontiguousarray((dt * b3).reshape(D, 1))
    in_maps = []
    for c in range(NCORES):
        y0T = np.ascontiguousarray(y0[c * BL:(c + 1) * BL].T)
        in_maps.append({"y0Tr": y0T.astype(bf16), "y0T32": y0T,
                        "w1": w1r.astype(bf16), "w2": w2r.astype(bf16),
                        "w3": w3r.astype(bf16),
                        "b1r": b1r, "b2r": b2r, "b3r": b3r})
    return in_maps


def kernel(y0, t, W1, b1, W2, b2, W3, b3, nsub: int = 2, nwarm: int = 0,
           act_relu2c1: bool = True, merged_upd: bool = False, **run_kwargs):
    nsteps = int(t.shape[0]) - 1
    key = (nsteps, nsub, nwarm, act_relu2c1, merged_upd)
    if key not in _cache:
        _cache[key] = build(nsteps, nsub, nwarm, act_relu2c1, merged_upd)
    nc = _cache[key]
    in_maps = _prep_inputs(y0, t, W1, b1, W2, b2, W3, b3)
    res = run_bass_kernel_spmd(nc, in_maps, core_ids=list(range(NCORES)),
                               **run_kwargs)
    parts = []
    for c in range(NCORES):
        oc = res.results[c]["out"]            # [nsteps, D, BL]
        parts.append(np.ascontiguousarray(oc.transpose(2, 0, 1)))  # [BL, ns, D]
    full = np.concatenate(parts, axis=0)      # [B, nsteps, D]
    out = np.concatenate([y0[:, None, :].astype(np.float32), full], axis=1)
    return out



# revision 9
# speedup vs baseline: 1.9585x; 1.9585x over previous
"""NeuralODE Euler-integration kernel for 8 TRN2 NeuronCores (v2).

Problem: y' = MLP(y), MLP = Linear(64,256)+ReLU, Linear(256,256)+ReLU,
Linear(256,64); Euler y_{t+1} = y_t + dt*MLP(y_t), T=200 (199 steps),
B=4096, output [B, 200, 64] with slot 0 = y0.

Sharding: pure data-parallel, 512 batch rows per core, weights replicated.

v2 design — keep the serial recurrence entirely on the PE:
  * a1 := W1^T y (pre-activation of layer 1) lives in PSUM across all
    steps. Identity: a1_{t+1} = a1_t + (dt*W3@W1)^T h2_t + dt*W1^T b3,
    realized as matmuls with start=False (PSUM has_written bits persist),
    so the critical loop is  MM(a1+=) -> relu(h1) -> MM(ps2) -> relu(h2)
    -> MM(a1+=)...  The Euler y-update never enters the chain.
  * ps_y := y/1 accumulates dt*W3^T h2 increments in another persistent
    PSUM bank (also start=False matmuls). One copy op per step moves
    ps_y -> SBUF (fp32, exact) for the DMA of y_t. No bf16 state shadow.
  * Batch split into S=2 independent streams of 256 cols; layers of the
    two streams are interleaved so PE/ACT/DVE always have work.
  * h1 is produced chunk-split (ACT does units 0:128, DVE 128:256, from
    separate PSUM banks -> parallel, low latency); h2 is produced by ONE
    merged op per stream ([128, 2, 256] from a single packed bank),
    alternating ACT/DVE, which minimizes fixed per-op overhead.
  * Init matmuls use float32r (exact fp32 at 1 cycle/col for N>=256):
    a1_0 = W1^T y0, ps_y_0 = I64 @ y0.
  * PE p-states ramp 0.65 -> 1.2 -> 2.4 GHz with sustained busy; nwarm0
    dummy matmuls before the loop + optional nfill per step keep it hot.

PSUM banks: a1 4 (one per stream x chunk), ps2 2 (mc chunks packed per
stream), ps_y 1 (both streams' columns), warm 1.
"""
import numpy as np

import concourse.bass as bass
import concourse.tile as tile
from concourse import bacc, mybir
from concourse.bass_utils import run_bass_kernel_spmd

F32 = mybir.dt.float32
F32R = mybir.dt.float32r
BF16 = mybir.dt.bfloat16
RELU = mybir.ActivationFunctionType.Relu
COPY = mybir.ActivationFunctionType.Copy

B, D, H, T = 4096, 64, 256, 200
NCORES = 8
BL = B // NCORES          # 512 batch rows per core
S = 2                     # independent batch streams per core
FD = BL // S              # 256 cols per stream

_cache = {}


def build(nsteps: int, nwarm0: int = 24, nfill: int = 0, has_b3: bool = False,
          b2_uniform: bool = True):
    add = mybir.AluOpType.add
    mx = mybir.AluOpType.max
    mult = mybir.AluOpType.mult

    nc = bacc.Bacc("TRN2", target_bir_lowering=False, debug=False)
    y0T_d = nc.dram_tensor("y0T", [D, BL], F32R, kind="ExternalInput")
    w1_d = nc.dram_tensor("w1", [D, 2, 128], F32R, kind="ExternalInput")
    i64_d = nc.dram_tensor("i64", [D, D], F32R, kind="ExternalInput")
    w31_d = nc.dram_tensor("w31", [128, 2, 2, 128], BF16, kind="ExternalInput")
    w2_d = nc.dram_tensor("w2", [128, 2, 2, 128], BF16, kind="ExternalInput")
    w3_d = nc.dram_tensor("w3", [128, 2, D], BF16, kind="ExternalInput")
    b1_d = nc.dram_tensor("b1r", [128, 2], F32, kind="ExternalInput")
    b2_d = nc.dram_tensor("b2r", [128, 2], F32, kind="ExternalInput")
    if has_b3:
        # c = dt*W1^T b3 (a1 increment), db3 = dt*b3 (ps_y increment)
        c_d = nc.dram_tensor("crow", [1, 2, 128], F32R, kind="ExternalInput")
        db3_d = nc.dram_tensor("db3row", [1, D], F32R, kind="ExternalInput")
    out_d = nc.dram_tensor("out", [nsteps, D, BL], F32, kind="ExternalOutput")

    with tile.TileContext(nc) as tc:
        with tc.tile_pool(name="wpool", bufs=1) as wp, \
             tc.tile_pool(name="hpool", bufs=8) as hp, \
             tc.tile_pool(name="ypool", bufs=4) as yp, \
             tc.tile_pool(name="ps", bufs=1, space="PSUM") as pp:

            y0T = wp.tile([D, BL], F32R)
            w1 = wp.tile([D, 2, 128], F32R)
            i64 = wp.tile([D, D], F32R)
            w31 = wp.tile([128, 2, 2, 128], BF16)
            w2 = wp.tile([128, 2, 2, 128], BF16)
            w3 = wp.tile([128, 2, D], BF16)
            b1 = wp.tile([128, 2], F32)
            b2 = wp.tile([128, 2], F32)
            nc.sync.dma_start(y0T[:], y0T_d.ap())
            nc.sync.dma_start(w1[:], w1_d.ap())
            nc.sync.dma_start(i64[:], i64_d.ap())
            nc.sync.dma_start(w31[:], w31_d.ap())
            nc.sync.dma_start(w2[:], w2_d.ap())
            nc.sync.dma_start(w3[:], w3_d.ap())
            nc.sync.dma_start(b1[:], b1_d.ap())
            nc.sync.dma_start(b2[:], b2_d.ap())
            if has_b3:
                crow = wp.tile([1, 2, 128], F32R)
                db3row = wp.tile([1, D], F32R)
                ones = wp.tile([1, FD], F32R)
                nc.sync.dma_start(crow[:], c_d.ap())
                nc.sync.dma_start(db3row[:], db3_d.ap())
                nc.gpsimd.memset(ones[:], 1.0)

            # Persistent PSUM state. a1 chunks get a full bank each so the
            # ACT/DVE h1 reads never share a bank.
            a1 = [[pp.tile([128, 2, FD], F32, tag=f"a1_{s}_{m}",
                           name=f"a1_{s}_{m}") for m in range(2)]
                  for s in range(S)]
            ps2 = [pp.tile([128, 2, FD], F32, tag=f"ps2_{s}", name=f"ps2_{s}")
                   for s in range(S)]
            psy = pp.tile([D, S, FD], F32, tag="psy", name="psy")
            wps = pp.tile([128, FD], F32, tag="warm", name="warm")

            out_ap = out_d.ap()
            warm_rhs = w2[:, 0, :, :].rearrange("p a b -> p (a b)")

            def fill(n):
                for _ in range(n):
                    nc.tensor.matmul(wps[:], w2[:, 0, 0, :], warm_rhs,
                                     start=True, stop=True,
                                     skip_group_check=True)

            fill(nwarm0)

            # ---- init: a1_0 = W1^T y0, ps_y_0 = y0 (fp32r, exact) ----
            for s in range(S):
                cs = bass.ts(s, FD)
                for mc in range(2):
                    nc.tensor.matmul(a1[s][mc][:, 0, :], w1[:, mc, :],
                                     y0T[:, cs], start=True, stop=True)
            # single start=True matmul for the whole psy bank: a second
            # start=True into the same bank would clear the first stream's
            # has_written bits and break the persistent accumulation
            nc.tensor.matmul(psy.rearrange("p a b -> p (a b)"), i64[:],
                             y0T[:], start=True, stop=True)

            h2p = [None] * S

            def emit_h1(t):
                h1s = []
                for s in range(S):
                    h1 = hp.tile([128, 2, FD], BF16, tag="h", name="h1")
                    nc.scalar.activation(h1[:, 0, :], a1[s][0][:, 0, :], RELU,
                                         bias=b1[:, 0:1], scale=1.0)
                    nc.vector.tensor_scalar(h1[:, 1, :], a1[s][1][:, 0, :],
                                            b1[:, 1:2], 0.0, op0=add, op1=mx)
                    h1s.append(h1)
                return h1s

            def emit_l2_h2(t, h1s):
                for s in range(S):
                    for mc in range(2):
                        for kc in range(2):
                            nc.tensor.matmul(ps2[s][:, mc, :],
                                             w2[:, kc, mc, :],
                                             h1s[s][:, kc, :],
                                             start=(kc == 0), stop=(kc == 1))
                for s in range(S):
                    h2 = hp.tile([128, 2, FD], BF16, tag="h", name="h2")
                    if b2_uniform:
                        # one merged op per stream; a single per-partition
                        # bias serves both chunks only when b2's two 128
                        # halves are identical (e.g. zero)
                        h2f = h2.rearrange("p a b -> p (a b)")
                        p2f = ps2[s].rearrange("p a b -> p (a b)")
                        if (t + s) % 2 == 0:
                            nc.scalar.activation(h2f, p2f, RELU,
                                                 bias=b2[:, 0:1], scale=1.0)
                        else:
                            nc.vector.tensor_scalar(h2f, p2f, b2[:, 0:1], 0.0,
                                                    op0=add, op1=mx)
                    else:
                        nc.scalar.activation(h2[:, 0, :], ps2[s][:, 0, :],
                                             RELU, bias=b2[:, 0:1], scale=1.0)
                        nc.vector.tensor_scalar(h2[:, 1, :], ps2[s][:, 1, :],
                                                b2[:, 1:2], 0.0,
                                                op0=add, op1=mx)
                    h2p[s] = h2

            # ---- t=0 hidden chain ----
            h1s0 = emit_h1(0)
            emit_l2_h2(0, h1s0)

            # ---- main loop ----
            for t in range(1, nsteps + 1):
                last = t == nsteps
                hprev = list(h2p)

                if not last:
                    for s in range(S):
                        for mc in range(2):
                            for kc in range(2):
                                nc.tensor.matmul(a1[s][mc][:, 0, :],
                                                 w31[:, kc, mc, :],
                                                 hprev[s][:, kc, :],
                                                 start=False, stop=(kc == 1),
                                                 skip_group_check=True)
                        if has_b3:
                            nc.tensor.matmul(a1[s][0][:, 0, :], crow[:, 0, :],
                                             ones[:], start=False, stop=True,
                                             skip_group_check=True)
                            nc.tensor.matmul(a1[s][1][:, 0, :], crow[:, 1, :],
                                             ones[:], start=False, stop=True,
                                             skip_group_check=True)
                    h1s = emit_h1(t)

                for s in range(S):
                    for kc in range(2):
                        nc.tensor.matmul(psy[:, s, :], w3[:, kc, :],
                                         hprev[s][:, kc, :],
                                         start=False, stop=(kc == 1),
                                         skip_group_check=True)
                    if has_b3:
                        nc.tensor.matmul(psy[:, s, :], db3row[:], ones[:],
                                         start=False, stop=True,
                                         skip_group_check=True)
                if nfill:
                    fill(nfill)

                if not last:
                    emit_l2_h2(t, h1s)

                yo = yp.tile([D, S, FD], F32, tag="yo", name="yo")
                yof = yo.rearrange("p a b -> p (a b)")
                pyf = psy.rearrange("p a b -> p (a b)")
                if t % 2 == 0:
                    nc.scalar.activation(yof, pyf, COPY)
                else:
                    nc.vector.tensor_scalar(yof, pyf, 1.0, 0.0,
                                            op0=mult, op1=add)
                nc.sync.dma_start(out_ap[t - 1], yof)
    nc.compile()
    return nc


def _prep_inputs(y0, t, W1, b1, W2, b2, W3, b3):
    import ml_dtypes
    bf16 = ml_dtypes.bfloat16
    f64 = np.float64
    dt = float(np.asarray(t)[1] - np.asarray(t)[0])

    w1r = np.ascontiguousarray(W1.reshape(D, 2, 128)).astype(np.float32)
    m31 = (dt * (W3.astype(f64) @ W1.astype(f64))).astype(np.float32)
    w31r = np.ascontiguousarray(
        m31.reshape(2, 128, 2, 128).transpose(1, 0, 2, 3))
    w2r = np.ascontiguousarray(W2.reshape(2, 128, 2, 128).transpose(1, 0, 2, 3))
    w3r = np.ascontiguousarray((dt * W3.astype(f64)).astype(np.float32)
                               .reshape(2, 128, D).transpose(1, 0, 2))
    b1r = np.ascontiguousarray(b1.reshape(2, 128).T).astype(np.float32)
    b2r = np.ascontiguousarray(b2.reshape(2, 128).T).astype(np.float32)
    i64 = np.eye(D, dtype=np.float32)

    has_b3 = bool(np.any(b3 != 0))
    b2_uniform = bool(np.array_equal(b2r[:, 0], b2r[:, 1]))
    crow = (dt * (W1.astype(f64).T @ b3.astype(f64))).astype(np.float32)
    crow = np.ascontiguousarray(crow.reshape(1, 2, 128))
    db3row = (dt * b3.astype(f64)).astype(np.float32).reshape(1, D)

    base = {"w1": w1r, "i64": i64,
            "w31": w31r.astype(bf16), "w2": w2r.astype(bf16),
            "w3": w3r.astype(bf16), "b1r": b1r, "b2r": b2r}
    if has_b3:
        base["crow"] = crow
        base["db3row"] = db3row
    in_maps = []
    for c in range(NCORES):
        y0T = np.ascontiguousarray(y0[c * BL:(c + 1) * BL].T).astype(np.float32)
        m = dict(base)
        m["y0T"] = y0T
        in_maps.append(m)
    return in_maps, has_b3, b2_uniform


def kernel(y0, t, W1, b1, W2, b2, W3, b3, nwarm0: int = 24, nfill: int = 0,
           **run_kwargs):
    nsteps = int(t.shape[0]) - 1
    in_maps, has_b3, b2_uniform = _prep_inputs(y0, t, W1, b1, W2, b2, W3, b3)
    key = (nsteps, nwarm0, nfill, has_b3, b2_uniform)
    if key not in _cache:
        _cache[key] = build(nsteps, nwarm0, nfill, has_b3, b2_uniform)
    nc = _cache[key]
    res = run_bass_kernel_spmd(nc, in_maps, core_ids=list(range(NCORES)),
                               **run_kwargs)
    parts = []
    for c in range(NCORES):
        oc = res.results[c]["out"]            # [nsteps, D, BL]
        parts.append(np.ascontiguousarray(oc.transpose(2, 0, 1)))  # [BL,ns,D]
    full = np.concatenate(parts, axis=0)      # [B, nsteps, D]
    out = np.concatenate([y0[:, None, :].astype(np.float32), full], axis=1)
    return out
